# revision 4
# baseline (speedup 1.0000x reference)
"""GNN message passing (out = A @ x, A[src,dst] = edge_attr) on 8 TRN2 NeuronCores.

Strategy: shard by destination segment (src rows of the output) across 8 cores.
Each core owns a contiguous 12,500-node src range and the edges whose src falls
in it. Per core:
  - edges are binned into (src-block [128 nodes], dst-bucket [25,000 nodes]) cells
  - a uniform chunk count per cell (max over cores) makes one static program
    serve all 8 cores (SPMD)
  - x rows are fetched with the SWDGE dma_gather custom op (bf16, 64B payload,
    256B stride, int16 indices local to the dst bucket)
  - DVE builds a one-hot scatter matrix per 128-edge chunk (iota == src_local)
    and folds edge weights into the gathered rows
  - PE matmul (one-hot^T @ messages) accumulates each src-block's [128, 32]
    output tile directly in PSUM across all of the block's chunks
  - one DVE copy PSUM->SBUF and one DMA writes the core's whole output
"""

import sys

sys.path.insert(0, "/opt/trn_rl_repo")

import numpy as np
import ml_dtypes

import concourse.bacc as bacc
import concourse.bass as bass
import concourse.mybir as mybir
from concourse.library_config import mlp
from concourse import bass_utils

N_NODES = 100000
D_FEAT = 32
N_CORES = 8
SRC_PER_CORE = N_NODES // N_CORES          # 12500
BLOCK = 128                                 # src nodes per block
N_BLOCKS = (SRC_PER_CORE + BLOCK - 1) // BLOCK   # 98
N_BUCKETS = 4
BUCKET = N_NODES // N_BUCKETS               # 25000 (fits int16 token index)
XPAD = 128                                  # bf16 row padded to 256B stride
NB = 64                                     # chunks per gather call / batch
NBUF = 3                                    # G/W buffer rotation depth


def _build_host_data(edge_index, edge_attr):
    src = np.asarray(edge_index[0], dtype=np.int64)
    dst = np.asarray(edge_index[1], dtype=np.int64)
    w = np.asarray(edge_attr, dtype=np.float32)
    E = src.shape[0]

    core = src // SRC_PER_CORE
    sloc = src % SRC_PER_CORE
    b = sloc // BLOCK
    srcl = sloc - b * BLOCK                  # 0..127 within block
    q = dst // BUCKET
    dstl = dst - q * BUCKET                  # 0..24999 within bucket

    # per (core, q, b) cell counts -> shared uniform chunk counts K[q, b]
    cell = (core * N_BUCKETS + q) * N_BLOCKS + b
    counts = np.bincount(cell, minlength=N_CORES * N_BUCKETS * N_BLOCKS)
    counts = counts.reshape(N_CORES, N_BUCKETS, N_BLOCKS)
    K = -(-counts.max(axis=0) // BLOCK)      # [N_BUCKETS, N_BLOCKS]
    K[0] = np.maximum(K[0], 1)               # every block writes its PSUM region

    chunk_start = np.zeros((N_BUCKETS, N_BLOCKS), dtype=np.int64)
    flat = K.reshape(-1)
    chunk_start.reshape(-1)[1:] = np.cumsum(flat)[:-1]
    C = int(flat.sum())

    # schedule metadata per chunk: bucket, block, start, stop
    chunk_q = np.repeat(np.arange(N_BUCKETS)[:, None], N_BLOCKS, 1).reshape(-1)
    chunk_q = np.repeat(chunk_q, flat)
    chunk_b = np.repeat(np.tile(np.arange(N_BLOCKS), N_BUCKETS), flat)
    is_start = np.zeros(C, dtype=bool)
    is_stop = np.zeros(C, dtype=bool)
    for bb in range(N_BLOCKS):
        own = np.where(chunk_b == bb)[0]
        is_start[own[0]] = True
        is_stop[own[-1]] = True

    # per-core slot assignment (slot = chunk*128 + lane)
    order = np.argsort(cell, kind="stable")
    cs = np.bincount(cell, minlength=N_CORES * N_BUCKETS * N_BLOCKS)
    cell_first = np.zeros_like(cs)
    cell_first[1:] = np.cumsum(cs)[:-1]
    rank = np.arange(E) - cell_first[cell[order]]
    slot_base = (chunk_start[q[order], b[order]] * BLOCK)
    slot = slot_base + rank                 # within this edge's core

    per_core = []
    dstl_o = dstl[order]
    srcl_o = srcl[order]
    w_o = w[order]
    core_o = core[order]
    for c in range(N_CORES):
        m = core_o == c
        s = slot[m]
        dl = np.zeros(C * BLOCK, dtype=np.int16)
        sl = np.zeros(C * BLOCK, dtype=np.int16)
        wv = np.zeros(C * BLOCK, dtype=np.float32)
        dl[s] = dstl_o[m].astype(np.int16)
        sl[s] = srcl_o[m].astype(np.int16)
        wv[s] = w_o[m]
        per_core.append((dl, sl, wv))

    # batches: per bucket, runs of <= NB chunks
    batches = []   # (q, cs_chunk, n_chunks)
    pos = 0
    for qq in range(N_BUCKETS):
        nq = int(K[qq].sum())
        done = 0
        while done < nq:
            n = min(NB, nq - done)
            batches.append((qq, pos + done, n))
            done += n
        pos += nq

    # wrapped int16 gather index arrays per core: [128, C*8]
    idx_w_cores = []
    for c in range(N_CORES):
        dl = per_core[c][0]
        cols = []
        for (qq, cs_c, n) in batches:
            flat_idx = dl[cs_c * BLOCK:(cs_c + n) * BLOCK]     # slot order == j order
            wrapped = flat_idx.reshape(-1, 16).T               # [16, ni/16]
            cols.append(np.tile(wrapped, (8, 1)))              # [128, ni/16]
        idx_w_cores.append(np.concatenate(cols, axis=1))

    sched = {
        "C": C,
        "chunk_b": chunk_b,
        "is_start": is_start,
        "is_stop": is_stop,
        "batches": batches,
    }
    return sched, per_core, idx_w_cores


def _dma_gather_raw(gpsimd, nc, out_ap, in_ap, idxs_ap, num_idxs, elem_size,
                    stride_bytes_256):
    """dma_gather with a sub-256B payload (elem_size*dtype < 256B) and an
    explicit 256B-multiple row stride. Same instruction the stock wrapper
    emits; the stock wrapper just over-asserts elem alignment."""
    _in_ap = gpsimd.lower_ap_dma(in_ap, for_custom_bir_dma=True)
    _idxs_ap = gpsimd.lower_ap(idxs_ap)
    _out_ap = gpsimd.lower_ap(out_ap)
    return gpsimd.add_instruction(
        mybir.InstDMAGatherAnt(
            name=nc.get_next_instruction_name(),
            ins=[*_in_ap, _idxs_ap, gpsimd.lower_val_access(gpsimd.to_reg(num_idxs))],
            outs=[_out_ap],
            transpose=False, num_idxs=num_idxs, elem_size=elem_size,
            stride_bytes_256=stride_bytes_256, gen_mode=0, single_packet=False,
            queue_num=0, sbuf_tokens_per_rank=0, sbuf_free_dim_per_rank=0,
            sbuf_free_dim_pad_per_rank=0, sbuf_byte_offset=0,
        )
    )


def _build_program(sched):
    C = sched["C"]
    chunk_b = sched["chunk_b"]
    is_start = sched["is_start"]
    is_stop = sched["is_stop"]
    batches = sched["batches"]
    nbatches = len(batches)
    OUTC = N_BLOCKS * D_FEAT                 # 3136

    bf16 = mybir.dt.bfloat16
    f32 = mybir.dt.float32

    nc = bacc.Bacc("TRN2", target_bir_lowering=False, debug=False,
                   num_devices=N_CORES)
    x_d = nc.dram_tensor("x", [N_NODES, XPAD], bf16, kind="ExternalInput")
    idx_d = nc.dram_tensor("idxw", [128, C * 8], mybir.dt.int16, kind="ExternalInput")
    srcl_d = nc.dram_tensor("srcl", [128, C], bf16, kind="ExternalInput")
    w_d = nc.dram_tensor("w", [128, C], bf16, kind="ExternalInput")
    iota_d = nc.dram_tensor("iota", [128, 128], bf16, kind="ExternalInput")
    out_d = nc.dram_tensor("out", [128, OUTC], f32, kind="ExternalOutput")

    with (
        nc.Block() as block,
        nc.sbuf_tensor("idx_sb", [128, C * 8], mybir.dt.int16) as idx_sb,
        nc.sbuf_tensor("srcl_sb", [128, C], bf16) as srcl_sb,
        nc.sbuf_tensor("w_sb", [128, C], bf16) as w_sb,
        nc.sbuf_tensor("iota_sb", [128, 128], bf16) as iota_sb,
        nc.sbuf_tensor("g_sb", [128, NBUF, NB * D_FEAT], bf16) as g_sb,
        nc.sbuf_tensor("wm_sb", [128, NBUF, NB * 128], bf16) as wm_sb,
        nc.sbuf_tensor("out_sb", [128, OUTC], f32) as out_sb,
        nc.psum_tensor("ps", [128, OUTC], f32) as ps,
        nc.semaphore("io") as io,
        nc.semaphore("gsem0") as gsem0,
        nc.semaphore("gsem1") as gsem1,
        nc.semaphore("gsem2") as gsem2,
        nc.semaphore("wsem") as wsem,
        nc.semaphore("psem") as psem,
        nc.semaphore("fin") as fin,
    ):
        @block.sync
        def _(sync):
            sync.dma_start(idx_sb[:], idx_d[:]).then_inc(io, 16)
            sync.dma_start(srcl_sb[:], srcl_d[:]).then_inc(io, 16)
            sync.dma_start(w_sb[:], w_d[:]).then_inc(io, 16)
            sync.dma_start(iota_sb[:], iota_d[:]).then_inc(io, 16)
            sync.wait_ge(fin, 1)
            sync.dma_start(out_d[:], out_sb[:]).then_inc(io, 16)
            sync.wait_ge(io, 80)

        @block.gpsimd
        def _(gpsimd):
            gpsimd.load_library(mlp)
            gpsimd.wait_ge(io, 64)  # all inputs loaded
            icol = 0
            for i, (qq, cs_c, n) in enumerate(batches):
                if i >= NBUF:
                    gpsimd.wait_ge(psem, i - NBUF + 1)
                ni = n * BLOCK
                buf = i % NBUF
                _dma_gather_raw(
                    gpsimd, nc,
                    out_ap=g_sb[:, buf, :n * D_FEAT].rearrange(
                        "p (n e) -> p n e", e=D_FEAT),
                    in_ap=x_d[qq * BUCKET:(qq + 1) * BUCKET, :D_FEAT],
                    idxs_ap=idx_sb[:, icol:icol + ni // 16],
                    num_idxs=ni, elem_size=D_FEAT,
                    stride_bytes_256=(XPAD * 2) // 256,
                ).then_inc([gsem0, gsem1, gsem2][buf], 16)
                icol += ni // 16

        @block.vector
        def _(vector):
            vector.memset(ps[:], 0.0).then_inc(wsem, 1)
            vector.wait_ge(io, 64)
            for i, (qq, cs_c, n) in enumerate(batches):
                buf = i % NBUF
                if i >= NBUF:
                    vector.wait_ge(psem, i - NBUF + 1)
                w3 = wm_sb[:, buf, :n * 128].rearrange("p (n s) -> p n s", s=128)
                vector.tensor_tensor(
                    out=w3,
                    in0=iota_sb[:, None, :].broadcast_to([128, n, 128]),
                    in1=srcl_sb[:, cs_c:cs_c + n, None].broadcast_to([128, n, 128]),
                    op=mybir.AluOpType.is_equal,
                ).then_inc(wsem, 1)
                vector.wait_ge([gsem0, gsem1, gsem2][buf], 16 * (i // NBUF + 1))
                g3 = g_sb[:, buf, :n * D_FEAT].rearrange("p (n e) -> p n e", e=D_FEAT)
                vector.tensor_tensor(
                    out=g3, in0=g3,
                    in1=w_sb[:, cs_c:cs_c + n, None].broadcast_to([128, n, D_FEAT]),
                    op=mybir.AluOpType.mult,
                ).then_inc(wsem, 1)
            vector.wait_ge(psem, nbatches)
            vector.tensor_copy(out=out_sb[:], in_=ps[:]).then_inc(fin, 1)

        @block.tensor
        def _(tensor):
            for i, (qq, cs_c, n) in enumerate(batches):
                buf = i % NBUF
                tensor.wait_ge(wsem, 2 * (i + 1) + 1)
                for k in range(n):
                    c = cs_c + k
                    off = int(chunk_b[c]) * D_FEAT
                    mm = nc.tensor.matmul(
                        out=ps[:, off:off + D_FEAT],
                        lhsT=wm_sb[:, buf, k * 128:(k + 1) * 128],
                        rhs=g_sb[:, buf, k * D_FEAT:(k + 1) * D_FEAT],
                        start=False, stop=False,
                        skip_group_check=True,
                    )
                mm.then_inc(psem, 1)


    nc.compile()
    return nc


def kernel(edge_index, edge_attr, x):
    sched, per_core, idx_w_cores = _build_host_data(edge_index, edge_attr)
    C = sched["C"]

    x_bf = np.zeros((N_NODES, XPAD), dtype=ml_dtypes.bfloat16)
    x_bf[:, :D_FEAT] = np.asarray(x, dtype=np.float32).astype(ml_dtypes.bfloat16)
    iota = np.tile(np.arange(128, dtype=np.float32).astype(ml_dtypes.bfloat16),
                   (128, 1))

    nc = _build_program(sched)

    in_maps = []
    for c in range(N_CORES):
        dl, sl, wv = per_core[c]
        in_maps.append({
            "x": x_bf,
            "idxw": idx_w_cores[c],
            "srcl": sl.reshape(C, BLOCK).T.astype(ml_dtypes.bfloat16).copy(),
            "w": wv.reshape(C, BLOCK).T.astype(ml_dtypes.bfloat16).copy(),
            "iota": iota,
        })

    res = bass_utils.run_bass_kernel_spmd(nc, in_maps, core_ids=list(range(N_CORES)))

    out = np.empty((N_NODES, D_FEAT), dtype=np.float32)
    for c in range(N_CORES):
        o = res.results[c]["out"]                      # [128, 98*32]
        o = o.reshape(128, N_BLOCKS, D_FEAT).transpose(1, 0, 2).reshape(-1, D_FEAT)
        out[c * SRC_PER_CORE:(c + 1) * SRC_PER_CORE] = o[:SRC_PER_CORE]
    return out
